# revision 1
# baseline (speedup 1.0000x reference)
"""Trainium2 Bass kernel for LocalScopeSelfAttention (3x3 window, clamp-padded).

Shapes (hardcoded): x [2, 8, 32, 32, 256] f32, 8 heads x hd=32, LN eps 1e-5.
Sharding: data-parallel over B*T=16 frames -> 2 frames per core on 8 cores.

v2: tensor-engine pipelining. The PE runs at stream rate (0.42ns/col) when the
instruction queue is free of dependency stalls, so the kernel is organized to
keep the tensor queue back-to-back:
  - all per-frame tensors are duplicated (frame-indexed) so frame f+1's
    preamble overlaps frame f's attention tail
  - the clamp-multiplicity mask is applied as a log-mask ACCUMULATING MATMUL
    into the scores psum (exp(s + ln m) == m * exp(s)), removing the
    elementwise mask multiply entirely
  - attention is software-pipelined: AV matmuls lag the scores matmuls by two
    subtiles so the scalar-engine exp has time to land
  - subtiles are paired in the AV psum ([128, 8, 33], odd subtile in
    partitions 64:128) halving the normalize/transpose work downstream
"""

import numpy as np
import ml_dtypes

H = W = 32
N = H * W          # 1024 tokens per frame
D = 256
NH, HD = 8, 32
LN_EPS = 1e-5
N_CORES = 8
FPC = 2            # frames per core
NPAD = N + 64      # padded tokens (32 guard each side)

_COMPILED = None


# ---------------------------------------------------------------- host helpers
def _build_lnmask_np():
    colcount = np.zeros((W, W), np.float32)
    for qc in range(W):
        for dc in (-1, 0, 1):
            colcount[qc, min(max(qc + dc, 0), W - 1)] += 1
    # rowcount[v][rq, rp] ; window rows are 2s-1 .. 2s+2 (rp = row - (2s-1))
    rowcounts = np.zeros((3, 2, 4), np.float32)
    for v, s in ((0, 0), (1, 7), (2, 15)):
        for rq in (0, 1):
            for dh in (-1, 0, 1):
                tgt = min(max(2 * s + rq + dh, 0), H - 1)
                rowcounts[v, rq, tgt - (2 * s - 1)] += 1
    masks = np.zeros((128, 3, 64), np.float32)
    for p in range(128):
        rp, kc = p // 32, p % 32
        for j in range(64):
            rq, qc = j // 32, j % 32
            for v in range(3):
                masks[p, v, j] = rowcounts[v, rq, rp] * colcount[qc, kc]
    lnm = np.where(masks > 0, np.log(np.maximum(masks, 1e-6)), -80.0)
    # repeat the [128, 64] block for all 8 (Q, g) head slots -> [128, 3, 512]
    lnm = np.tile(lnm[:, :, None, :], (1, 1, 8, 1)).reshape(128, 3, 512)
    return lnm.astype(ml_dtypes.bfloat16)


def _fold_params(inp):
    f32 = np.float32
    g = inp["ln_g"].astype(f32)
    lb = inp["ln_b"].astype(f32)
    s = f32(1.0 / np.sqrt(HD))
    wq = (g[:, None] * inp["wq"].astype(f32)) * s
    bq = (lb @ inp["wq"].astype(f32) + inp["bq"].astype(f32)) * s
    wk = g[:, None] * inp["wk"].astype(f32)
    wv = g[:, None] * inp["wv"].astype(f32)
    bv = lb @ inp["wv"].astype(f32) + inp["bv"].astype(f32)
    wo = inp["wo"].astype(f32)
    bo = bv @ wo + inp["bo"].astype(f32)
    bf = ml_dtypes.bfloat16
    # weight sbuf layout [128, kc, m]: w[kc*128+p, m]
    def wfmt(w):
        return np.ascontiguousarray(w.reshape(2, 128, 256).transpose(1, 0, 2)).astype(bf)
    return {
        "wq": wfmt(wq), "wk": wfmt(wk), "wv": wfmt(wv), "wo": wfmt(wo),
        "bq": bq.reshape(1, 256).astype(bf),
        "bo": bo.reshape(1, 256).astype(bf),
        "lnm": _build_lnmask_np(),
    }


# ---------------------------------------------------------------- bass build
def _build_bass():
    from contextlib import ExitStack
    import concourse.tile as tile
    from concourse import bacc, mybir

    dt = mybir.dt
    AF = mybir.ActivationFunctionType
    OP = mybir.AluOpType

    nc = bacc.Bacc("TRN2", target_bir_lowering=False, debug=False,
                   num_devices=N_CORES)

    x_d = nc.dram_tensor("x", [FPC * N, D], dt.float32, kind="ExternalInput").ap()
    wq_d = nc.dram_tensor("wq", [128, 2, 256], dt.bfloat16, kind="ExternalInput").ap()
    wk_d = nc.dram_tensor("wk", [128, 2, 256], dt.bfloat16, kind="ExternalInput").ap()
    wv_d = nc.dram_tensor("wv", [128, 2, 256], dt.bfloat16, kind="ExternalInput").ap()
    wo_d = nc.dram_tensor("wo", [128, 2, 256], dt.bfloat16, kind="ExternalInput").ap()
    bq_d = nc.dram_tensor("bq", [1, 256], dt.bfloat16, kind="ExternalInput").ap()
    bo_d = nc.dram_tensor("bo", [1, 256], dt.bfloat16, kind="ExternalInput").ap()
    lnm_d = nc.dram_tensor("lnm", [128, 3, 512], dt.bfloat16, kind="ExternalInput").ap()
    y_d = nc.dram_tensor("y", [FPC * N, D], dt.float32, kind="ExternalOutput").ap()

    with tile.TileContext(nc) as tc:
        with ExitStack() as ctx:
            const = ctx.enter_context(tc.tile_pool(name="const", bufs=1))
            frame = ctx.enter_context(tc.tile_pool(name="frame", bufs=1))
            work = ctx.enter_context(tc.tile_pool(name="work", bufs=3))
            att = ctx.enter_context(tc.tile_pool(name="att", bufs=4))
            pp = ctx.enter_context(tc.tile_pool(name="pp", bufs=2, space="PSUM"))
            pst = ctx.enter_context(tc.tile_pool(name="pst", bufs=3, space="PSUM"))
            pav = ctx.enter_context(tc.tile_pool(name="pav", bufs=2, space="PSUM"))
            ptp = ctx.enter_context(tc.tile_pool(name="ptp", bufs=1, space="PSUM"))

            # ---- constants ----
            wq_s = const.tile([128, 2, 256], dt.bfloat16)
            wk_s = const.tile([128, 2, 256], dt.bfloat16)
            wv_s = const.tile([128, 2, 256], dt.bfloat16)
            wo_s = const.tile([128, 2, 256], dt.bfloat16)
            for sb, d in ((wq_s, wq_d), (wk_s, wk_d), (wv_s, wv_d), (wo_s, wo_d)):
                nc.scalar.dma_start(sb[:], d[:])
            bq_s = const.tile([1, 256], dt.bfloat16)
            bo_s = const.tile([1, 256], dt.bfloat16)
            lnm_s = const.tile([128, 3, 512], dt.bfloat16)
            nc.scalar.dma_start(bq_s[:], bq_d[:])
            nc.scalar.dma_start(bo_s[:], bo_d[:])
            nc.scalar.dma_start(lnm_s[:], lnm_d[:])
            ones_s = const.tile([1, 1024], dt.bfloat16)
            nc.vector.memset(ones_s[:], 1.0)
            ident = const.tile([128, 128], dt.bfloat16)
            from concourse.masks import make_identity
            make_identity(nc, ident[:])
            for cval in (0.0, LN_EPS):
                ct = const.tile([128, 1], dt.float32, tag=f"c{cval}")
                nc.vector.memset(ct[:], cval)
                nc.const_aps.aps[(dt.float32, cval)] = ct[:]

            # ---- per-frame persistent tensors, duplicated over FPC ----
            def ftiles(shape, dtype, nm):
                return [frame.tile(shape, dtype, name=f"{nm}{f}")
                        for f in range(FPC)]

            xnT = ftiles([128, 2, NPAD], dt.bfloat16, "xnT")
            kTp = ftiles([128, 2, NPAD], dt.bfloat16, "kTp")
            qst = frame.tile([128, 2, 4, N], dt.bfloat16, name="qst")
            vau = ftiles([128, 9, NH, 33], dt.bfloat16, "vau")
            vau64 = ftiles([128, 8, NH, 33], dt.bfloat16, "vau64")
            xoT = ftiles([128, 2, N], dt.bfloat16, "xoT")
            x_f = ftiles([128, 8, 256], dt.float32, "x_f")
            mv = ftiles([128, 8, 2], dt.float32, "mv")
            rstd = ftiles([128, 8], dt.float32, "rstd")
            lnv = ftiles([128, 8], dt.float32, "lnv")

            # qst is shared by both frames: stripes are rewritten per frame,
            # the inter-stripe zeros are written once here (split across two
            # engines; DMA-pool stays free for the x loads)
            nc.vector.memset(qst[:, 0], 0.0)
            nc.gpsimd.memset(qst[:, 1], 0.0)
            # one-time zero/one fills (pads persist across frames)
            for f in range(FPC):
                nc.vector.memset(xnT[f][:, :, 0:32], 0.0)
                nc.vector.memset(xnT[f][:, :, 32 + N:], 0.0)
                nc.vector.memset(kTp[f][:, :, 0:32], 0.0)
                nc.vector.memset(kTp[f][:, :, 32 + N:], 0.0)
                # vau: ones only where the v casts do not write -- the
                # denominator column 32 of every head, and the tail rows of
                # chunk 8. vau64 needs no fill: its two DMAs cover all of it.
                nc.vector.memset(vau[f][:, :, :, 32], 1.0)
                nc.vector.memset(vau[f][64:128, 8], 1.0)

            def pe_filler(n):
                for _ in range(n):
                    dmy = pst.tile([128, 2, 4, 64], dt.float32,
                                   tag="pst", name="dmy")
                    nc.tensor.matmul(
                        dmy[:].rearrange("p q g j -> p (q g j)"),
                        wq_s[:, 0, 0:128], lnm_s[:, 1, :],
                        start=True, stop=True)

            pe_filler(4)
            for f in range(FPC):
                for half in range(2):
                    nc.sync.dma_start(
                        x_f[f][:, 4 * half:4 * half + 4, :],
                        x_d[f * N + 512 * half:f * N + 512 * (half + 1), :]
                        .rearrange("(b p) d -> p b d", p=128))

            def ln_bn(f, i):
                st = work.tile([128, 6], dt.float32, tag="bnst", name="st")
                nc.vector.bn_stats(st[:], x_f[f][:, i, :])
                nc.vector.bn_aggr(mv[f][:, i, :], st[:])

            def ln_rstd(f, half):
                hs = slice(4 * half, 4 * half + 4)
                nc.scalar.activation(lnv[f][:, hs], mv[f][:, hs, 1],
                                     AF.Sqrt, bias=LN_EPS, scale=1.0)
                nc.vector.reciprocal(rstd[f][:, hs], lnv[f][:, hs])

            def ln_xnt(f, i):
                xn = work.tile([128, 256], dt.bfloat16, tag="xn", name="xn")
                nc.vector.tensor_scalar(
                    xn[:], x_f[f][:, i, :], mv[f][:, i, 0:1], rstd[f][:, i:i + 1],
                    OP.subtract, OP.mult)
                ptj = ptp.tile([128, 256], dt.bfloat16, tag="ptj", name="ptj")
                ptr = ptj[:, 0:256]
                for kc in range(2):
                    nc.tensor.transpose(
                        ptr[:, 128 * kc:128 * (kc + 1)],
                        xn[:, 128 * kc:128 * (kc + 1)], ident[:])
                nc.vector.tensor_copy(
                    xnT[f][:, :, 32 + 128 * i:32 + 128 * (i + 1)],
                    ptr[:].rearrange("p (k t) -> p k t", k=2))

            for f in range(FPC):
                xf_dram = x_d[f * N:(f + 1) * N, :]
                # ---------------- LN ----------------
                # frame 1's LN was pre-warmed inside frame 0's attention loop
                if f == 0:
                    for half in range(2):
                        for i in range(4 * half, 4 * half + 4):
                            ln_bn(f, i)
                        ln_rstd(f, half)
                        for i in range(4 * half, 4 * half + 4):
                            ln_xnt(f, i)
                            # filler matmuls keep the PE busy while the vector
                            # LN chain trickles, holding the DVFS ramp
                            pe_filler(1)

                # ---------------- q projection -> qst stripes ----------------
                for nh in range(2):
                    for mc in range(2):
                        ns = slice(512 * nh, 512 * (nh + 1))
                        pq = pp.tile([128, 512], dt.float32, tag="pp")
                        for kc in range(2):
                            nc.tensor.matmul(
                                pq[:], wq_s[:, kc, 128 * mc:128 * (mc + 1)],
                                xnT[f][:, kc, 32 + 512 * nh:32 + 512 * (nh + 1)],
                                start=(kc == 0), stop=False)
                        nc.tensor.matmul(
                            pq[:], bq_s[0:1, 128 * mc:128 * (mc + 1)],
                            ones_s[0:1, ns], start=False, stop=True)
                        qn = work.tile([128, 512], dt.bfloat16, tag="qn")
                        nc.scalar.copy(qn[:], pq[:])
                        for g in range(4):
                            eng = nc.gpsimd if g % 2 == 0 else nc.sync
                            eng.dma_start(
                                qst[32 * g:32 * (g + 1), mc, g, ns],
                                qn[32 * g:32 * (g + 1), :])

                # ---------------- k projection -> kTp ----------------
                for nh in range(2):
                    for mc in range(2):
                        pk = pp.tile([128, 512], dt.float32, tag="pp")
                        for kc in range(2):
                            nc.tensor.matmul(
                                pk[:], wk_s[:, kc, 128 * mc:128 * (mc + 1)],
                                xnT[f][:, kc, 32 + 512 * nh:32 + 512 * (nh + 1)],
                                start=(kc == 0), stop=(kc == 1))
                        nc.vector.tensor_copy(
                            kTp[f][:, mc, 32 + 512 * nh:32 + 512 * (nh + 1)], pk[:])

                # ---------------- v projection (pad-aligned chunks) ----------
                for c in range(9):
                    np_ = 128 if c < 8 else 64
                    pvv = pav.tile([128, NH, 33], dt.float32, tag="pav", name="pvv")
                    for kc in range(2):
                        nc.tensor.matmul(
                            pvv[0:np_, :, 0:32],
                            xnT[f][:, kc, 128 * c:128 * c + np_],
                            wv_s[:, kc, :],
                            start=(kc == 0), stop=(kc == 1))
                    nc.scalar.copy(vau[f][0:np_, c, :, 0:32], pvv[0:np_, :, 0:32])
                nc.gpsimd.dma_start(vau64[f][0:64], vau[f][64:128, 0:8])
                nc.gpsimd.dma_start(vau64[f][64:128, 0:8], vau[f][0:64, 1:9])
                pe_filler(2)

                # ---------------- attention: software-pipelined subtiles -----
                # slot t: scores(t) | AV(t-2) | pair-tail((t-5)//2)
                psts = {}
                aes = {}
                pas = {}

                def em_scores(s):
                    pst_t = pst.tile([128, 2, 4, 64], dt.float32, tag="pst", name="pst_t")
                    psts[s] = pst_t
                    vi = 0 if s == 0 else (2 if s == 15 else 1)
                    nc.tensor.matmul(
                        pst_t[:].rearrange("p q g j -> p (q g j)"),
                        ident[:], lnm_s[:, vi, :], start=True, stop=False)
                    for Q in range(2):
                        nc.tensor.matmul(
                            pst_t[:, Q], kTp[f][:, Q, 64 * s:64 * s + 128],
                            qst[:, Q, :, 64 * s:64 * s + 64],
                            start=False, stop=(Q == 1), skip_group_check=True)

                def em_exp(s):
                    ae = att.tile([128, NH, 64], dt.bfloat16, tag="ae", name="ae")
                    aes[s] = ae
                    nc.scalar.activation(
                        ae[:], psts[s][:].rearrange("p q g j -> p (q g) j"),
                        AF.Exp, bias=0.0, scale=1.0)

                def em_av(s):
                    if s % 2 == 0:
                        pa = pav.tile([128, NH, 33], dt.float32, tag="pav", name="pa")
                        pas[s // 2] = pa
                    pa = pas[s // 2]
                    qs = slice(0, 64) if s % 2 == 0 else slice(64, 128)
                    vsrc = vau[f][:, s // 2] if s % 2 == 0 else vau64[f][:, (s - 1) // 2]
                    for h in range(NH):
                        nc.tensor.matmul(
                            pa[qs, h, :], aes[s][:, h, :], vsrc[:, h, :],
                            start=True, stop=True)

                ptjs = {}

                def em_pair_T(p):
                    pa = pas[p]
                    rc = att.tile([128, NH], dt.float32, tag="rc", name="rc")
                    nc.vector.reciprocal(rc[:], pa[:, :, 32])
                    on2 = att.tile([128, NH, 32], dt.bfloat16, tag="on2", name="on2")
                    nc.vector.tensor_tensor(
                        on2[:], pa[:, :, 0:32],
                        rc[:].unsqueeze(2).to_broadcast((128, NH, 32)), OP.mult)
                    on2v = on2[:].rearrange("p h c -> p (h c)")
                    ptj = ptp.tile([128, 256], dt.bfloat16, tag="ptj", name="ptj")
                    ptjs[p] = ptj
                    ptr = ptj[:, 0:256]
                    for kc in range(2):
                        nc.tensor.transpose(
                            ptr[:, 128 * kc:128 * (kc + 1)],
                            on2v[:, 128 * kc:128 * (kc + 1)], ident[:])
                    nc.vector.tensor_copy(
                        xoT[f][:, :, 128 * p:128 * (p + 1)],
                        ptr[:].rearrange("p (k t) -> p k t", k=2))

                def em_pair_O(p):
                    # out projection + residual + store for token block p
                    pyt = pp.tile([128, 512], dt.float32, tag="pp", name="pyt")
                    py = pyt[:, 0:256]
                    for kc in range(2):
                        nc.tensor.matmul(
                            py[:], xoT[f][:, kc, 128 * p:128 * (p + 1)],
                            wo_s[:, kc, :], start=(kc == 0), stop=False)
                    nc.tensor.matmul(
                        py[:], ones_s[0:1, 0:128], bo_s[:], start=False, stop=True)
                    ys = work.tile([128, 256], dt.float32, tag="ys", name="ys")
                    nc.vector.tensor_tensor(ys[:], py[:], x_f[f][:, p, :], OP.add)
                    nc.sync.dma_start(
                        y_d[f * N + 128 * p:f * N + 128 * (p + 1), :], ys[:])

                prewarm = {}
                if f == 0:
                    prewarm = {
                        4: [lambda: ln_bn(1, 0), lambda: ln_bn(1, 1)],
                        5: [lambda: ln_bn(1, 2), lambda: ln_bn(1, 3)],
                        6: [lambda: ln_rstd(1, 0)],
                        7: [lambda: ln_xnt(1, 0)],
                        9: [lambda: ln_xnt(1, 1)],
                        11: [lambda: ln_xnt(1, 2), lambda: ln_bn(1, 4),
                             lambda: ln_bn(1, 5)],
                        12: [lambda: ln_bn(1, 6), lambda: ln_bn(1, 7)],
                        13: [lambda: ln_xnt(1, 3), lambda: ln_rstd(1, 1)],
                        15: [lambda: ln_xnt(1, 4)],
                        17: [lambda: ln_xnt(1, 5)],
                        19: [lambda: ln_xnt(1, 6)],
                        21: [lambda: ln_xnt(1, 7)],
                    }
                for t in range(23):
                    if t < 16:
                        em_scores(t)
                    else:
                        # tail slots: hold the PE clock through the
                        # latency-bound final pairs
                        pe_filler(1)
                    if 1 <= t < 17:
                        em_exp(t - 1)
                    if 2 <= t < 18:
                        em_av(t - 2)
                    if t >= 5 and (t - 5) % 2 == 0 and (t - 5) // 2 < 8:
                        em_pair_T((t - 5) // 2)
                    if t >= 7 and (t - 7) % 2 == 0 and (t - 7) // 2 < 8:
                        em_pair_O((t - 7) // 2)
                    for fn in prewarm.get(t, ()):
                        fn()

    nc.compile()
    return nc


# ---------------------------------------------------------------- entry point
def kernel(**inputs):
    global _COMPILED
    if _COMPILED is None:
        _COMPILED = _build_bass()
    nc = _COMPILED

    from concourse.bass_utils import run_bass_kernel_spmd

    x = np.asarray(inputs["x"], dtype=np.float32)          # [2, 8, 32, 32, 256]
    B, T = x.shape[0], x.shape[1]
    frames = x.reshape(B * T, N, D)
    params = _fold_params({k: np.asarray(v) for k, v in inputs.items()})

    in_maps = []
    for c in range(N_CORES):
        m = {"x": np.ascontiguousarray(
            frames[FPC * c:FPC * (c + 1)].reshape(FPC * N, D))}
        m.update(params)
        in_maps.append(m)

    res = run_bass_kernel_spmd(nc, in_maps, list(range(N_CORES)))
    y = np.concatenate([res.results[c]["y"].reshape(FPC, N, D)
                        for c in range(N_CORES)], axis=0)
    return y.reshape(x.shape).astype(np.float32)

